# revision 4
# baseline (speedup 1.0000x reference)
"""LSTMCell Bass kernel for 8 Trainium2 NeuronCores.

Problem: B=2048, IN=1024, H=1024 LSTM cell, fp32.
    gi = concat(x, hidden)           # [B, 2048]
    g_k = gi @ W_k + b_k             # 4 gates, W_k [2048, 1024]
    c' = sig(g_f)*c + sig(g_i)*tanh(g_c);  h' = sig(g_o)*tanh(c')

Sharding (8 cores): 2 batch blocks (Bc=1024) x 4 hidden blocks (Hc=256).
Per-core matmul: [Bc,2048] @ [2048, 4*Hc] = 4.3 GFLOP, computed transposed
(out[H,B] = W^T stationary, gi^T moving) so gate bias is per-partition and
all elementwise work stays in [H,B] layout. Matmul dtype float32r (fp22
multiply, fp32 accumulate): 1 cyc/row vs 4 for fp32, rel err ~1e-4.

Host pre-lays out every array partition-major so each DMA is contiguous
per partition. Outputs return as (c_next, h_next), matching the reference.
"""

import numpy as np

import concourse.bass as bass
import concourse.mybir as mybir
import concourse.tile as tile
from concourse import bacc
from concourse.bass_utils import run_bass_kernel_spmd

F32 = mybir.dt.float32
F32R = mybir.dt.float32r
AF = mybir.ActivationFunctionType

B, IN, H = 2048, 1024, 1024
K = IN + H            # 2048 contraction
NB, NH = 2, 4         # batch blocks x hidden blocks = 8 cores
BC, HC = B // NB, H // NH   # 1024, 256 per core
KC = K // 128         # 16 k-chunks
M = 4 * HC // 128     # 8 output row chunks (hc in {0,1} x gate in {f,i,c,o})
NN = BC // 512        # 2 moving-side chunks of 512

_CACHE = {}


def _build_module():
    nc = bacc.Bacc("TRN2", target_bir_lowering=False, debug=False)

    gi_d = nc.dram_tensor("gi", [128, KC * NN * 512], F32R, kind="ExternalInput").ap()
    w_d = nc.dram_tensor("w", [128, KC * M * 128], F32R, kind="ExternalInput").ap()
    cell_d = nc.dram_tensor("cellT", [128, 2 * NN * 512], F32, kind="ExternalInput").ap()
    b_d = nc.dram_tensor("bT", [128, M], F32, kind="ExternalInput").ap()
    out_d = nc.dram_tensor("out", [128, 2 * 2 * NN * 512], F32, kind="ExternalOutput").ap()

    with tile.TileContext(nc) as tc:
        with (
            tc.tile_pool(name="const", bufs=1) as const_pool,
            tc.tile_pool(name="cell", bufs=2) as cell_pool,
            tc.tile_pool(name="giP", bufs=KC * NN) as gi_pool,
            tc.tile_pool(name="wP", bufs=KC) as w_pool,
            tc.tile_pool(name="psum", bufs=8, space="PSUM") as psum_pool,
            tc.tile_pool(name="gates", bufs=2) as gate_pool,
            tc.tile_pool(name="tmp", bufs=2) as tmp_pool,
            tc.tile_pool(name="outs", bufs=2) as out_pool,
        ):
            bias_t = const_pool.tile([128, M], F32)
            nc.sync.dma_start(bias_t[:], b_d[:])

            cell_t = []
            for hc in range(2):
                ct = cell_pool.tile([128, NN * 512], F32)
                nc.sync.dma_start(ct[:], cell_d[:, hc * NN * 512:(hc + 1) * NN * 512])
                cell_t.append(ct)

            gi_t = {}
            w_t = {}
            psum = {}

            for n in range(NN):
                for kc in range(KC):
                    if n == 0:
                        wt = w_pool.tile([128, M * 128], F32R, name=f"w{kc}", tag="w")
                        nc.sync.dma_start(
                            wt[:], w_d[:, kc * M * 128:(kc + 1) * M * 128])
                        w_t[kc] = wt
                    gt = gi_pool.tile([128, 512], F32R, name=f"gi{kc}_{n}", tag="gi")
                    nc.sync.dma_start(
                        gt[:], gi_d[:, (kc * NN + n) * 512:(kc * NN + n + 1) * 512])
                    gi_t[(kc, n)] = gt

                    for m in range(M):
                        if kc == 0:
                            psum[(m, n)] = psum_pool.tile([128, 512], F32, name=f"ps{m}_{n}", tag="ps")
                        nc.tensor.matmul(
                            psum[(m, n)][:],
                            lhsT=w_t[kc][:, m * 128:(m + 1) * 128],
                            rhs=gi_t[(kc, n)][:],
                            start=(kc == 0),
                            stop=(kc == KC - 1),
                        )

                # combine: gates -> c', h' for each hc row-chunk of this n
                for hc in range(2):
                    mf, mi, mc, mo = hc * 4, hc * 4 + 1, hc * 4 + 2, hc * 4 + 3
                    f_t = gate_pool.tile([128, 512], F32)
                    nc.scalar.activation(f_t[:], psum[(mf, n)][:], AF.Sigmoid,
                                         bias=bias_t[:, mf:mf + 1])
                    i_t = gate_pool.tile([128, 512], F32)
                    nc.scalar.activation(i_t[:], psum[(mi, n)][:], AF.Sigmoid,
                                         bias=bias_t[:, mi:mi + 1])
                    ch_t = gate_pool.tile([128, 512], F32)
                    nc.scalar.activation(ch_t[:], psum[(mc, n)][:], AF.Tanh,
                                         bias=bias_t[:, mc:mc + 1])
                    o_t = gate_pool.tile([128, 512], F32)
                    nc.scalar.activation(o_t[:], psum[(mo, n)][:], AF.Sigmoid,
                                         bias=bias_t[:, mo:mo + 1])

                    cell_sl = cell_t[hc][:, n * 512:(n + 1) * 512]
                    t1 = tmp_pool.tile([128, 512], F32)
                    nc.vector.tensor_mul(t1[:], f_t[:], cell_sl)
                    t2 = tmp_pool.tile([128, 512], F32)
                    nc.vector.tensor_mul(t2[:], i_t[:], ch_t[:])
                    c_t = out_pool.tile([128, 512], F32)
                    nc.vector.tensor_add(c_t[:], t1[:], t2[:])
                    nc.sync.dma_start(
                        out_d[:, ((0 * 2 + hc) * NN + n) * 512:
                                 ((0 * 2 + hc) * NN + n + 1) * 512], c_t[:])
                    tanh_c = tmp_pool.tile([128, 512], F32)
                    nc.scalar.activation(tanh_c[:], c_t[:], AF.Tanh)
                    h_t = out_pool.tile([128, 512], F32)
                    nc.vector.tensor_mul(h_t[:], o_t[:], tanh_c[:])
                    nc.sync.dma_start(
                        out_d[:, ((1 * 2 + hc) * NN + n) * 512:
                                 ((1 * 2 + hc) * NN + n + 1) * 512], h_t[:])

    nc.compile()
    return nc


def _prep_inputs(x, cell_state, hidden, W_f, b_f, W_i, b_i, W_c, b_c, W_o, b_o):
    """Per-core partition-major layouts. Core id = ib * NH + jh."""
    gi = np.concatenate([x, hidden], axis=1)  # [B, K]
    Ws = [W_f, W_i, W_c, W_o]
    bs = [b_f, b_i, b_c, b_o]
    in_maps = []
    for ib in range(NB):
        gi_sl = gi[ib * BC:(ib + 1) * BC, :]  # [BC, K]
        # gi_arr[p, kc, n, b] = gi_sl[n*512 + b, kc*128 + p]
        gi_arr = np.ascontiguousarray(
            gi_sl.reshape(NN, 512, KC, 128).transpose(3, 2, 0, 1)
        ).reshape(128, KC * NN * 512)
        for jh in range(NH):
            # W_cat [K, M, 128]: m = hc*4 + g
            cols = []
            for hc in range(2):
                for g in range(4):
                    c0 = jh * HC + hc * 128
                    cols.append(Ws[g][:, c0:c0 + 128])
            W_core = np.stack(cols, axis=1)  # [K, M, 128]
            w_arr = np.ascontiguousarray(
                W_core.reshape(KC, 128, M, 128).transpose(1, 0, 2, 3)
            ).reshape(128, KC * M * 128)

            b_core = np.stack(
                [bs[g][jh * HC + hc * 128: jh * HC + (hc + 1) * 128]
                 for hc in range(2) for g in range(4)], axis=0)  # [M, 128]
            b_arr = np.ascontiguousarray(b_core.T)  # [128, M]

            cell_sl = cell_state[ib * BC:(ib + 1) * BC, jh * HC:(jh + 1) * HC]
            # cellT[p, hc, n, b] = cell_sl[n*512+b, hc*128+p]
            cell_arr = np.ascontiguousarray(
                cell_sl.reshape(NN, 512, 2, 128).transpose(3, 2, 0, 1)
            ).reshape(128, 2 * NN * 512)

            in_maps.append({
                "gi": gi_arr, "w": w_arr, "cellT": cell_arr, "bT": b_arr,
            })
    return in_maps


def _unpack_outputs(results):
    c_next = np.empty((B, H), np.float32)
    h_next = np.empty((B, H), np.float32)
    for ib in range(NB):
        for jh in range(NH):
            o = results[ib * NH + jh]["out"].reshape(128, 2, 2, NN, 512)
            # o[p, which, hc, n, b] -> out[n*512+b, hc*128+p]
            blk_c = o[:, 0].transpose(2, 3, 1, 0).reshape(BC, HC)
            blk_h = o[:, 1].transpose(2, 3, 1, 0).reshape(BC, HC)
            c_next[ib * BC:(ib + 1) * BC, jh * HC:(jh + 1) * HC] = blk_c
            h_next[ib * BC:(ib + 1) * BC, jh * HC:(jh + 1) * HC] = blk_h
    return c_next, h_next


def _run(in_maps, trace=False, trace_cores=None):
    if "nc" not in _CACHE:
        _CACHE["nc"] = _build_module()
    res = run_bass_kernel_spmd(
        _CACHE["nc"], in_maps, core_ids=list(range(NB * NH)),
        trace=trace, trace_cores=trace_cores,
    )
    return res


def kernel(x, cell_state, hidden, W_f, b_f, W_i, b_i, W_c, b_c, W_o, b_o):
    args = [np.ascontiguousarray(np.asarray(a, np.float32)) for a in
            (x, cell_state, hidden, W_f, b_f, W_i, b_i, W_c, b_c, W_o, b_o)]
    in_maps = _prep_inputs(*args)
    res = _run(in_maps)
    return _unpack_outputs(res.results)
